# revision 32
# baseline (speedup 1.0000x reference)
"""Causal self-attention (B=2, S=2048, E=1024, H=16, D=64) on 8 trn2 NeuronCores.

Sharding: core c = (batch b = c // 4, head-group g = c % 4).  Each core computes
4 heads (one quarter of the 16) for one batch: projections q/k/v for its 256
output channels, then causal flash-style attention, writing out[b, :, 256g:256g+256].

Per-core kernel design (Bass/Tile), v2:
  - Host pre-transposes hidden -> hT [E, S] (bf16, t4 groups ascending) and
    weight slices -> wT [E, 256] (bf16) so matmul contractions have K on
    partitions.
  - PE clock warmup: a short stream of scratch matmuls at t=0 flips the HAM
    clock gate to 8/8 before real work lands (DMA-bound prologue is idle
    anyway).
  - Ascending-j schedule: attention blocks consume q/k/v levels in streaming
    order so the first block starts after ~2.5MB of DMA instead of ~5.5MB.
  - q/k projections (PSUM-accumulated over 8 E-chunks) produce qT/kT in [d, t]
    layout (f32->bf16 with scale 1/8 on q, bias add).
  - v projection produces v in [t, d]; DVE splits head pairs into vaug tiles
    [tk=128, 2, 65] with a ones column per head (sum-of-exp trick).
  - scores^T tiles [tk=128, tq=512] per head; the two heads of a pair run
    concurrently on PE 64-row tiles (T0/T8, auto via base_partition).
  - exp via ScalarE activation (attention-mask bias per tk partition), bf16.
  - causal masking: DVE tensor_mul by precomputed staircase mask tiles
    (built once on gpsimd in the prologue) - keeps gpsimd latency out of the
    exp->attnv chain.
  - attn @ vaug accumulates unnormalized out^T [65, tq] in PSUM; row 64 is
    the softmax denominator.
  - norm: gpsimd copies PSUM->SBUF u2 [65, 1024], PE transposes into one
    [128, 520] PSUM tile, single DVE reciprocal [128, 8], two DVE muls
    write bf16 [t, d] output tiles; DMA to DRAM (bf16, host upcasts).
"""

import numpy as np
import ml_dtypes

import concourse.bass as bass
import concourse.mybir as mybir
import concourse.tile as tile
from concourse import bacc
from concourse.bass_utils import run_bass_kernel_spmd

F32 = mybir.dt.float32
BF16 = mybir.dt.bfloat16

B, S, E = 2, 2048, 1024
H, D = 16, 64
NCORES = 8
OC = 256          # output channels per core (4 heads)
NPAIR = 2         # head pairs per core
NT = S // 128     # 16 tk tiles
NT4 = S // 512    # 4 tq blocks

_cached_nc = None


def _patch_ldw_opt():
    # walrus is invoked with --enable-ldw-opt=false hardcoded; LDWEIGHTS
    # scheduling opt measurably tightens back-to-back matmul spacing.
    import os
    if os.environ.get("LDW_OPT", "0") != "1":
        return
    import concourse.bass_utils as _bu
    if getattr(_bu, "_ldw_patched", False):
        return
    _orig = _bu.run_command

    def _patched(argv, **kw):
        argv = ["--enable-ldw-opt=true" if a == "--enable-ldw-opt=false" else a
                for a in argv]
        return _orig(argv, **kw)

    _bu.run_command = _patched
    _bu._ldw_patched = True


def _build():
    _patch_ldw_opt()
    nc = bacc.Bacc()

    hT = nc.declare_dram_parameter("hT", [128, 32 * 512], BF16, isOutput=False)
    wqT = nc.declare_dram_parameter("wqT", [128, 2048], BF16, isOutput=False)
    wkT = nc.declare_dram_parameter("wkT", [128, 2048], BF16, isOutput=False)
    wvT = nc.declare_dram_parameter("wvT", [128, 2048], BF16, isOutput=False)
    bqp = nc.declare_dram_parameter("bqp", [128, 2], F32, isOutput=False)
    bkp = nc.declare_dram_parameter("bkp", [128, 2], F32, isOutput=False)
    bvf = nc.declare_dram_parameter("bvf", [OC], F32, isOutput=False)
    mask_t = nc.declare_dram_parameter("mask_t", [128, NT], F32, isOutput=False)
    ident = nc.declare_dram_parameter("ident", [65, 65], BF16, isOutput=False)
    out = nc.declare_dram_parameter("out", [S, OC], BF16, isOutput=True)

    EXP = mybir.ActivationFunctionType.Exp
    ADD = mybir.AluOpType.add
    MULT = mybir.AluOpType.mult
    GE = mybir.AluOpType.is_ge

    with tile.TileContext(nc) as tc:
        with (
            tc.tile_pool(name="cst", bufs=1) as cst,
            tc.tile_pool(name="work", bufs=4) as work,
            tc.tile_pool(name="expp", bufs=5) as expp,
            tc.tile_pool(name="ps_small", bufs=1, space="PSUM") as ps_small,
            tc.tile_pool(name="ps_tp", bufs=1, space="PSUM") as ps_tp,
            tc.tile_pool(name="ps_sc", bufs=2, space="PSUM") as ps_sc,
            tc.tile_pool(name="ps_out", bufs=1, space="PSUM") as ps_out,
        ):
            # ---- PE clock warmup: scratch matmuls on a zeroed tile keep the
            # HAM activity window busy during the DMA-bound prologue so the
            # clock gate opens at ~3.5us instead of ~23us. Results unused. ----
            wz = cst.tile([128, 512], BF16, tag="wz")
            nc.gpsimd.memset(wz, 0.0)
            ps_warm = ps_tp.tile([128, 512], F32, tag="tp", name="ps_warm")
            for _ in range(8):
                nc.tensor.matmul(ps_warm, wz[:, 0:128], wz,
                                 start=True, stop=True)

            # ---- big resident inputs: host-packed in consumption order
            # (t4 groups ascending; wq/wk pair-major so pair-0 slices land
            # first; e-chunks side by side). ----
            hT_big = cst.tile([128, 32 * 512], BF16, tag="hT_big")
            wq_big = cst.tile([128, 2048], BF16, tag="wq_big")
            wk_big = cst.tile([128, 2048], BF16, tag="wk_big")
            wv_big = cst.tile([128, 2048], BF16, tag="wv_big")
            nc.sync.dma_start(out=wq_big[:, 0:1024], in_=wqT[:, 0:1024])
            bq_sb = cst.tile([128, 2], F32, tag="bq")
            nc.sync.dma_start(out=bq_sb, in_=bqp[:, :])
            bk_sb = cst.tile([128, 2], F32, tag="bk")
            nc.sync.dma_start(out=bk_sb, in_=bkp[:, :])
            mask_sb = cst.tile([128, NT], F32, tag="mask")
            nc.sync.dma_start(out=mask_sb, in_=mask_t[:, :])
            nc.sync.dma_start(out=hT_big[:, 0:2048], in_=hT[:, 0:2048])
            nc.sync.dma_start(out=hT_big[:, 2048:4096], in_=hT[:, 2048:4096])
            nc.sync.dma_start(out=wk_big[:, 0:1024], in_=wkT[:, 0:1024])
            nc.sync.dma_start(out=wv_big[:, 0:1024], in_=wvT[:, 0:1024])
            nc.sync.dma_start(out=wv_big[:, 1024:2048], in_=wvT[:, 1024:2048])
            bv_sb = cst.tile([128, OC], F32, tag="bv")
            nc.gpsimd.dma_start(out=bv_sb, in_=bvf[:].partition_broadcast(128))
            nc.sync.dma_start(out=wq_big[:, 1024:2048], in_=wqT[:, 1024:2048])
            nc.sync.dma_start(out=wk_big[:, 1024:2048], in_=wkT[:, 1024:2048])
            ident_sb = cst.tile([65, 65], BF16, tag="ident")
            nc.sync.dma_start(out=ident_sb, in_=ident[:, :])
            nc.sync.dma_start(out=hT_big[:, 4096:6144], in_=hT[:, 4096:6144])
            nc.sync.dma_start(out=hT_big[:, 6144:8192], in_=hT[:, 6144:8192])
            nc.sync.dma_start(out=hT_big[:, 8192:12288], in_=hT[:, 8192:12288])
            nc.sync.dma_start(out=hT_big[:, 12288:16384], in_=hT[:, 12288:16384])

            # ---- causal staircase masks, one per s-offset (s = 128*m).
            # mk[m][r, h, f] = 1.0 if f >= 128*m + r else 0.0, f in [0,512).
            # Built once on gpsimd (idle in the prologue); applied by DVE
            # tensor_mul after exp, replacing in-chain gpsimd affine_select.
            mk_all = cst.tile([128, 4096], BF16, tag="mk")
            nc.vector.memset(mk_all, 1.0)
            mk = []
            for m in range(4):
                mt3 = mk_all[:, 1024 * m:1024 * (m + 1)].rearrange(
                    "a (h f) -> a h f", h=2)
                nc.gpsimd.affine_select(
                    out=mt3, in_=mt3, compare_op=GE, fill=0.0,
                    base=-(128 * m), pattern=[[0, 2], [1, 512]],
                    channel_multiplier=-1,
                )
                mk.append(mt3)

            hT32 = [[hT_big[:, t4 * 4096 + e * 512: t4 * 4096 + (e + 1) * 512]
                     for t4 in range(NT4)] for e in range(8)]
            # q/k weights pair-major: [p, e, 128]; v e-major: [e, 256]
            wqk = {"q": wq_big, "k": wk_big}
            wv_sb = [wv_big[:, e * OC:(e + 1) * OC] for e in range(8)]

            # ---- persistent intermediates ----
            qT = [cst.tile([128, S], BF16, tag=f"qT{p}", name=f"qT{p}") for p in range(NPAIR)]
            kT = [cst.tile([128, S], BF16, tag=f"kT{p}", name=f"kT{p}") for p in range(NPAIR)]
            va_big = [cst.tile([128, NT * 130], BF16, tag=f"va{p}", name=f"va{p}")
                      for p in range(NPAIR)]
            vaug = [[va_big[p][:, 130 * tt:130 * (tt + 1)]
                     for tt in range(NT)] for p in range(NPAIR)]
            outsb_all = cst.tile([128, NT * OC], BF16, tag="outsb_all", name="outsb_all")
            out_tt = outsb_all.rearrange("a (tt c) -> a tt c", tt=NT)
            # per-j dram view iterated (partition, s4, col) to match the
            # SBUF staging layout [128, s4*OC + c]
            out4v = out.rearrange("(f s a) c -> f a s c", s=4, a=128)
            out4 = [out4v[jj] for jj in range(NT4)]

            def emit_qk_chain(nm, p, t4):
                po = 128 * p
                dst = qT[p] if nm == "q" else kT[p]
                b_sb = bq_sb if nm == "q" else bk_sb
                ts = slice(512 * t4, 512 * (t4 + 1))
                ps_qk = ps_small.tile([128, 512], F32, tag="sm", name="ps_qk")
                for e in range(8):
                    nc.tensor.matmul(
                        ps_qk,
                        wqk[nm][:, 1024 * p + 128 * e:1024 * p + 128 * (e + 1)],
                        hT32[e][t4],
                        start=(e == 0), stop=(e == 7),
                    )
                if nm == "q":
                    nc.vector.tensor_scalar(
                        out=dst[:, ts], in0=ps_qk,
                        scalar1=0.125, scalar2=b_sb[:, p:p + 1],
                        op0=MULT, op1=ADD,
                    )
                else:
                    nc.vector.tensor_scalar_add(
                        out=dst[:, ts], in0=ps_qk, scalar1=b_sb[:, p:p + 1],
                    )

            def emit_v_chain(tt):
                t4v, r4 = divmod(tt, 4)
                rs = slice(128 * r4, 128 * (r4 + 1))
                ps_v = ps_small.tile([128, OC], F32, tag="sm", name="ps_v")
                for e in range(8):
                    nc.tensor.matmul(
                        ps_v,
                        hT32[e][t4v][:, rs],
                        wv_sb[e][:, :],
                        start=(e == 0), stop=(e == 7),
                    )
                for p in range(NPAIR):
                    po = 128 * p
                    vt3 = vaug[p][tt].rearrange("a (h c) -> a h c", h=2)
                    ps3 = ps_v[:, po:po + 128].rearrange("a (h c) -> a h c", h=2)
                    bv3 = bv_sb[:, po:po + 128].rearrange("a (h c) -> a h c", h=2)
                    nc.vector.tensor_add(vt3[:, :, 0:64], ps3, bv3)
                    nc.vector.memset(vt3[:, :, 64:65], 1.0)

            chores_q = []

            def emit_attn_block(p, j):
                po = 128 * p
                out_AB = ps_out.tile([65, 1024], F32, tag="out", name="out_AB")
                u2 = work.tile([65, 1024], BF16, tag="u", name="u2")
                tpb = ps_tp.tile([128, 528], BF16, tag="tp", name="tpb")
                ntk = 4 * (j + 1)
                pend_tp = []

                def emit_tp(s4):
                    for h_loc in range(2):
                        g = 4 * h_loc + s4
                        nc.tensor.transpose(
                            tpb[:, 66 * g:66 * g + 65],
                            u2[:, 512 * h_loc + 128 * s4:512 * h_loc + 128 * (s4 + 1)],
                            ident_sb)

                for i in range(ntk):
                    ks = slice(128 * i, 128 * (i + 1))
                    crossing = i >= 4 * j
                    # valid tq columns of this tile start at s (cols < s are
                    # entirely above the diagonal): trim scores/exp/attn@v
                    s = 128 * i - 512 * j if crossing else 0
                    qsv = slice(512 * j + s, 512 * (j + 1))
                    sc = ps_sc.tile([128, 1024], F32, tag="sc", name="sc")
                    nc.tensor.matmul(sc[:, s:512], kT[p][0:64, ks],
                                     qT[p][0:64, qsv], start=True, stop=True)
                    nc.tensor.matmul(sc[:, 512 + s:1024], kT[p][64:128, ks],
                                     qT[p][64:128, qsv], start=True, stop=True)
                    ex = expp.tile([128, 1024], BF16, tag="exp", name="ex")
                    ex3 = ex.rearrange("a (h f) -> a h f", h=2)
                    if s:
                        exv = ex3[:, :, s:512]
                        scv = sc.rearrange("a (h f) -> a h f", h=2)[:, :, s:512]
                    else:
                        exv, scv = ex, sc
                    nc.scalar.activation(out=exv, in_=scv, func=EXP,
                                         bias=mask_sb[:, i:i + 1], scale=1.0)
                    if crossing:
                        m = s // 128
                        nc.vector.tensor_mul(
                            ex3[:, :, s:512], ex3[:, :, s:512],
                            mk[m][:, :, s:512])
                    va3 = vaug[p][i].rearrange("a (h c) -> a h c", h=2)
                    nc.tensor.matmul(out_AB[:, s:512], va3[:, 0, :],
                                     ex[:, s:512],
                                     start=(i == 0), stop=(i == ntk - 1))
                    nc.tensor.matmul(out_AB[:, 512 + s:1024], va3[:, 1, :],
                                     ex[:, 512 + s:1024],
                                     start=(i == 0), stop=(i == ntk - 1))
                    # transposes for the quarter finalized last iteration
                    # (its PSUM->SBUF copy has had a full iteration to land)
                    for tr in pend_tp:
                        tr()
                    pend_tp = []
                    if crossing:
                        # query quarter s4 of out_AB got its last
                        # accumulation: copy it out now (alternating DVE /
                        # ScalarE), transpose next iteration
                        s4 = i - 4 * j
                        u23 = u2.rearrange("a (h f) -> a h f", h=2)
                        o3 = out_AB.rearrange("a (h f) -> a h f", h=2)
                        cq = slice(128 * s4, 128 * (s4 + 1))
                        nc.vector.tensor_copy(u23[:, :, cq], o3[:, :, cq])
                        pend_tp.append(lambda s4=s4: emit_tp(s4))
                    if chores_q:
                        c = chores_q.pop(0)
                        if c is not None:
                            c()

                for tr in pend_tp:
                    tr()
                tp3 = tpb.rearrange("a (g c) -> a g c", g=8)
                r = work.tile([128, 8], F32, tag="r", name="r")
                nc.vector.reciprocal(
                    r, tp3[:, :, 64:65].rearrange("a g c -> a (g c)"))
                r3 = r.rearrange("a (h s o) -> a h s o", h=2, o=1)
                for h_loc in range(2):
                    c0 = po + 64 * h_loc
                    nc.vector.tensor_mul(
                        out_tt[:, 4 * j:4 * (j + 1), c0:c0 + 64],
                        tp3[:, 4 * h_loc:4 * h_loc + 4, 0:64],
                        r3[:, h_loc].broadcast_to([128, 4, 64]))
                if p == NPAIR - 1:
                    # one DMA for the whole 512-query block: iteration order
                    # (partition, s4, col) on both sides
                    nc.sync.dma_start(
                        out=out4[j], in_=outsb_all[:, 4 * j * OC:4 * (j + 1) * OC])

            # ---- schedule: levels ascending for DMA streaming; chores give
            # the PE fill-in work while attention is ScalarE-bound.  The
            # small (1,0) block runs last for a short tail. ----
            # chore schedule: one slot per attention i-iteration, in block
            # emission order.  Late-deadline chains are deferred into the
            # ScalarE-bound late blocks (slots 33+) so the PE-bound early
            # window sheds work.  Slot numbering: (0,0)=1-4, (0,1)=5-12,
            # (1,1)=13-20, (0,2)=21-32, (1,2)=33-44, (0,3)=45-60,
            # (1,0)=61-64, (1,3)=65-80.
            emit_qk_chain("q", 0, 0)
            emit_qk_chain("k", 0, 0)
            for tt in range(4):
                emit_v_chain(tt)
            C = chores_q.append
            C(lambda: emit_qk_chain("k", 1, 0))      # 1
            C(lambda: emit_qk_chain("q", 0, 1))      # 2
            C(lambda: emit_qk_chain("k", 0, 1))      # 3
            C(lambda: emit_v_chain(4))               # 4
            C(lambda: emit_v_chain(5))               # 5
            C(lambda: emit_v_chain(6))               # 6
            C(lambda: emit_v_chain(7))               # 7
            C(lambda: emit_qk_chain("q", 1, 1))      # 8
            C(lambda: emit_qk_chain("k", 1, 1))      # 9
            C(None)                                  # 10
            C(None)                                  # 11
            C(None)                                  # 12
            C(lambda: emit_qk_chain("q", 0, 2))      # 13 (needed slot 21)
            C(lambda: emit_qk_chain("k", 0, 2))      # 14
            for _ in range(6):                       # 15-20
                C(None)
            C(lambda: emit_v_chain(8))               # 21 (consumed slot 29)
            C(lambda: emit_v_chain(9))               # 22
            C(lambda: emit_v_chain(10))              # 23
            C(lambda: emit_v_chain(11))              # 24
            C(lambda: emit_qk_chain("q", 1, 2))      # 25 (needed slot 33)
            C(lambda: emit_qk_chain("k", 1, 2))      # 26
            for _ in range(6):                       # 27-32
                C(None)
            C(lambda: emit_qk_chain("q", 0, 3))      # 33 (needed slot 45)
            C(lambda: emit_qk_chain("k", 0, 3))      # 34
            C(lambda: emit_v_chain(12))              # 35 (consumed slot 57)
            C(lambda: emit_v_chain(13))              # 36
            for _ in range(8):                       # 37-44
                C(None)
            C(lambda: emit_v_chain(14))              # 45 (consumed slot 59)
            C(lambda: emit_v_chain(15))              # 46
            C(lambda: emit_qk_chain("q", 1, 3))      # 47 (needed slot 65)
            C(lambda: emit_qk_chain("k", 1, 3))      # 48
            C(lambda: emit_qk_chain("q", 1, 0))      # 49 (needed slot 61)
            for p, j in ((0, 0), (0, 1), (1, 1), (0, 2), (1, 2), (0, 3),
                         (1, 0), (1, 3)):
                emit_attn_block(p, j)

    nc.compile()
    return nc


def _get_nc():
    global _cached_nc
    if _cached_nc is None:
        _cached_nc = _build()
    return _cached_nc


def make_in_maps(hidden_states, attention_mask, Wq, bq, Wk, bk, Wv, bv):
    hidden_states = np.asarray(hidden_states, dtype=np.float32)
    attention_mask = np.asarray(attention_mask, dtype=np.float32)
    Wq = np.asarray(Wq, dtype=np.float32)
    Wk = np.asarray(Wk, dtype=np.float32)
    Wv = np.asarray(Wv, dtype=np.float32)
    bq = np.asarray(bq, dtype=np.float32)
    bk = np.asarray(bk, dtype=np.float32)
    bv = np.asarray(bv, dtype=np.float32)

    bf = ml_dtypes.bfloat16
    ident = np.eye(65, dtype=np.float32).astype(bf)
    in_maps = []
    for c in range(NCORES):
        b, g = divmod(c, 4)
        cs = slice(OC * g, OC * (g + 1))
        hTT = np.ascontiguousarray(hidden_states[b].T).astype(bf)  # [E, S]
        hp = np.empty((128, 32 * 512), dtype=bf)
        for t4 in range(4):
            for e in range(8):
                hp[:, t4 * 4096 + e * 512:t4 * 4096 + (e + 1) * 512] = \
                    hTT[e * 128:(e + 1) * 128, t4 * 512:(t4 + 1) * 512]

        def packw(W):
            # e-major: [e, 256] (used for v)
            wT = np.ascontiguousarray(W[cs, :].T).astype(bf)  # [E, 256]
            wp = np.empty((128, 2048), dtype=bf)
            for e in range(8):
                wp[:, e * OC:(e + 1) * OC] = wT[e * 128:(e + 1) * 128, :]
            return wp

        def packw_pair(W):
            # pair-major: [p, e, 128] so pair-0 slices are contiguous first
            wT = np.ascontiguousarray(W[cs, :].T).astype(bf)  # [E, 256]
            wp = np.empty((128, 2048), dtype=bf)
            for p in range(2):
                for e in range(8):
                    wp[:, 1024 * p + 128 * e:1024 * p + 128 * (e + 1)] = \
                        wT[e * 128:(e + 1) * 128, 128 * p:128 * (p + 1)]
            return wp

        in_maps.append({
            "hT": hp,
            "wqT": packw_pair(Wq),
            "wkT": packw_pair(Wk),
            "wvT": packw(Wv),
            "bqp": np.ascontiguousarray(bq[cs].reshape(2, 128).T),
            "bkp": np.ascontiguousarray(bk[cs].reshape(2, 128).T),
            "bvf": np.ascontiguousarray(bv[cs]),
            "mask_t": np.ascontiguousarray(
                attention_mask[b, 0, 0, :].reshape(NT, 128).T),
            "ident": ident,
        })
    return in_maps


def kernel(hidden_states, attention_mask, Wq, bq, Wk, bk, Wv, bv):
    in_maps = make_in_maps(hidden_states, attention_mask,
                           Wq, bq, Wk, bk, Wv, bv)
    nc = _get_nc()
    res = run_bass_kernel_spmd(nc, in_maps, list(range(NCORES)))

    full = np.empty((B, S, H * D), dtype=np.float32)
    for c in range(NCORES):
        b, g = divmod(c, 4)
        full[b, :, OC * g:OC * (g + 1)] = \
            np.asarray(res.results[c]["out"]).astype(np.float32)
    return full
